# revision 1
# baseline (speedup 1.0000x reference)
# Multi-head attention (B=2, S=4096, D=512, H=8) on 8 Trainium2 NeuronCores.
#
# Sharding: core c handles batch b=c//4 and query rows [(c%4)*1024, (c%4+1)*1024).
# Each core computes K/V projections for its full batch element (duplicated
# across the 4 cores sharing a batch; avoids all cross-core communication),
# Q for its own slice, full 8-head attention for its query rows, and the
# output projection for its rows. The full output is the disjoint
# concatenation of the 8 per-core results.
#
# On-core dataflow (matmuls bf16 with fp32 PSUM accumulation):
#   x,W --HWDGE load--> SBUF f32 --Pool/DVE cast--> SBUF bf16
#     --PE transpose (identity built via iota)--> x^T, W^T tiles in SBUF
#   (no DRAM round-trip, no DMA-transpose: the PE transposes 128x128 blocks
#   into half-bank bf16 PSUM tiles that share the projection PSUM ring)
#   Q^T = Wq^T-tiles x x^T   K^T = Wk^T-tiles x x^T   V = x^T-tiles x Wv^T
#   V is stored with a ones-column appended per head (65 cols/head), so the
#   PV matmul produces both O^T (64 rows) and the softmax denominator l
#   (row 64) in one instruction -- no separate ones-matmul.
#   per (q-chunk 512, head-pair j), per k-tile kt of 128:
#     S^T[k,q] for BOTH heads -> one 2-bank PSUM tile   (2 row-packed MMs)
#     P^T = exp(S^T * scale): one ACT instruction for both heads, or a
#       Schraudolph bitcast-exp on DVE for a subset of k-tiles (splits the
#       exp throughput across two engines; the softmax denominator uses the
#       same approximated P so the ~3% sawtooth error cancels to 1st order)
#     O^T[0:65,q] += Vx-tile^T x P^T  per head (separate PSUM banks, m=65)
#   PV for k-tile kt is emitted PV_LAG slots behind its scores so the PE
#   never stalls behind the exp engines in its FIFO.
#   pair end: DVE copies O^T+l out of PSUM (frees banks fast), DVE reciprocal
#   of l, broadcast of 1/l across partitions via a small DRAM bounce, DVE
#   mul -> OT (bf16); head B's rows move to partitions 64:128 via SBUF DMA.
#   y = OT-tiles x Wout^T + b  (consumes OT directly as stationary operand)
import sys

if "/opt/trn_rl_repo" not in sys.path:
    sys.path.insert(0, "/opt/trn_rl_repo")

import numpy as np

B = 2
S = 4096
DIM = 512
H = 8
DH = DIM // H
SCALE = DH**-0.5
N_CORES = 8
QLOC = S // 4  # query rows per core
N_KT = S // 128  # k tiles of 128
N_DT = DIM // 128  # feature-dim tiles of 128
PV_LAG = 4  # slots between scores+exp emission and the PV matmul
# Slots (of 32 per pair) whose exp runs on DVE (Schraudolph bitcast exp).
DVE_SLOTS = frozenset({2, 5, 9, 12, 16, 19, 23, 26, 29})
EXP_A = 12102203.161561485  # 2**23 / ln(2)
EXP_B = 1064986823.0  # 127 * 2**23 - 366393 (minimax shift)

_CACHE = {}


def _build_program(reps=1):
    from contextlib import ExitStack

    from concourse import bacc, mybir, tile

    f32 = mybir.dt.float32
    bf16 = mybir.dt.bfloat16
    i32 = mybir.dt.int32
    Exp = mybir.ActivationFunctionType.Exp

    nc = bacc.Bacc("TRN2", target_bir_lowering=False, debug=False)

    x_full = nc.dram_tensor("x_full", [S, DIM], f32, kind="ExternalInput")
    x_q = nc.dram_tensor("x_q", [QLOC, DIM], f32, kind="ExternalInput")
    w_qkv = nc.dram_tensor("w_qkv", [3 * DIM, DIM], f32, kind="ExternalInput")
    w_out = nc.dram_tensor("w_out", [DIM, DIM], f32, kind="ExternalInput")
    b_out = nc.dram_tensor("b_out", [1, DIM], f32, kind="ExternalInput")
    y_out = nc.dram_tensor("y", [QLOC, DIM], f32, kind="ExternalOutput")

    # per-pair 1/l rows bounced through DRAM to broadcast across partitions
    rb_dr = nc.dram_tensor("rb_dr", [8, 2, 512], f32)

    with tile.TileContext(nc) as tc, ExitStack() as ctx:
        if reps > 1:  # benchmarking only: repeat the whole body in a HW loop
            ctx.enter_context(tc.For_i(0, reps, 1))
        consts = ctx.enter_context(tc.tile_pool(name="consts", bufs=1))
        wp = ctx.enter_context(tc.tile_pool(name="wp", bufs=1))
        big = ctx.enter_context(tc.tile_pool(name="big", bufs=1))
        xtp = ctx.enter_context(tc.tile_pool(name="xtp", bufs=2))
        ptp = ctx.enter_context(tc.tile_pool(name="ptp", bufs=8))
        otp = ctx.enter_context(tc.tile_pool(name="otp", bufs=1))
        obp = ctx.enter_context(tc.tile_pool(name="obp", bufs=2))
        rbp = ctx.enter_context(tc.tile_pool(name="rbp", bufs=2))
        ysp = ctx.enter_context(tc.tile_pool(name="ysp", bufs=2))
        castp = ctx.enter_context(tc.tile_pool(name="castp", bufs=3))
        # PSUM: proj/transpose ring 2 banks + sAB 2x2 banks + otA + otB = 8
        pp = ctx.enter_context(tc.tile_pool(name="pp", bufs=2, space="PSUM"))
        sp = ctx.enter_context(tc.tile_pool(name="sp", bufs=2, space="PSUM"))
        op = ctx.enter_context(tc.tile_pool(name="op", bufs=1, space="PSUM"))

        bias_sb = consts.tile([128, DIM], f32, tag="bias")
        nc.gpsimd.dma_start(out=bias_sb[:], in_=b_out.ap().broadcast_to([128, DIM]))

        # identity for PE-mode transposes, built on-chip: (f - p) == 0
        it32 = consts.tile([128, 128], i32, tag="it32")
        nc.gpsimd.iota(it32[:], pattern=[[1, 128]], base=0, channel_multiplier=-1)
        ident = consts.tile([128, 128], bf16, tag="ident")
        nc.gpsimd.tensor_scalar(
            ident[:], it32[:], 0, None, mybir.AluOpType.is_equal
        )

        # --- persistent per-core tensors ---
        KT = big.tile([128, N_DT, S], bf16, tag="KT")  # K^T: part=(e-512)%128, [et, s]
        # V with a ones column appended per head: [k%128, kt, h*65+(dh|64)]
        Vx = big.tile([128, N_KT, H * (DH + 1)], bf16, tag="Vx")
        QT = big.tile([128, N_DT, QLOC], bf16, tag="QT")  # Q^T: part=e%128, [et, q]
        wqkvT = wp.tile([128, N_DT, 3 * DIM], bf16, tag="wqkvT")
        woutT = wp.tile([128, N_DT, DIM], bf16, tag="woutT")

        # --- load + cast (f32->bf16) a 512-row chunk into SBUF ---
        cast_n = [0]

        def load_cast(src_dram, r0, q=None, eng=None, rows=512):
            cast_n[0] += 1
            a = rows // 128
            xf = castp.tile([128, a, DIM], f32, tag="castf", name=f"cf{cast_n[0]}")
            (q or nc.sync).dma_start(
                out=xf[:],
                in_=src_dram.ap()[r0 : r0 + rows, :].rearrange(
                    "(a p) d -> p a d", p=128
                ),
            )
            xb = castp.tile([128, a, DIM], bf16, tag="castb", name=f"cb{cast_n[0]}")
            (eng or nc.gpsimd).tensor_copy(xb[:], xf[:])
            return xb

        # --- PE-transpose a cast chunk into a [128, dt, 512] T-layout tile ---
        # dst columns c0+a*128+c get rows r0+a*128+p of the source chunk.
        def transpose_chunk(dst, cb, c0=0, a0=0, na=4):
            for dt2 in range(N_DT // 2):
                tp = pp.tile([128, 2, 512], bf16, tag="proj", name=f"tp{cast_n[0]}{dt2}")
                for u in range(2):
                    dt = 2 * dt2 + u
                    for a in range(na):
                        nc.tensor.transpose(
                            tp[:, u, a * 128 : (a + 1) * 128],
                            cb[:, a, dt * 128 : (dt + 1) * 128],
                            ident[:],
                        )
                if na == 4:  # contiguous dst: one fused evac frees the ring faster
                    nc.vector.tensor_copy(
                        dst[:, 2 * dt2 : 2 * dt2 + 2, c0 : c0 + 512], tp[:]
                    )
                else:
                    for u in range(2):
                        dt = 2 * dt2 + u
                        nc.vector.tensor_copy(
                            dst[:, dt, c0 + a0 * 128 : c0 + (a0 + na) * 128],
                            tp[:, u, 0 : na * 128],
                        )

        def qproj(qc, xqT):
            for et in range(N_DT):
                ps = pp.tile([128, 512], f32, tag="proj")
                for dt in range(N_DT):
                    nc.tensor.matmul(
                        ps[:],
                        wqkvT[:, dt, et * 128 : (et + 1) * 128],
                        xqT[:, dt, :],
                        start=(dt == 0),
                        stop=(dt == N_DT - 1),
                    )
                nc.vector.tensor_copy(QT[:, et, qc * 512 : (qc + 1) * 512], ps[:])

        # --- attention machinery ---
        pair_state = {}

        def pair_begin(qc, j):
            otA = op.tile([128, 512], f32, tag="otA", name=f"otA{qc}{j}")
            otB = op.tile([128, 512], f32, tag="otB", name=f"otB{qc}{j}")
            pair_state[(qc, j)] = [otA, otB, []]

        def emit_pv(qc, j, slot, pH):
            # slot = 2*g + hb: head hb of pair j, k-tiles {2g, 2g+1}.
            # One fp8 DoubleRow matmul contracts both k-tiles and emits
            # O^T (rows 0:64) plus the softmax denominator (row 64).
            otA, otB, _ = pair_state[(qc, j)]
            g, hb = slot // 2, slot % 2
            h = 2 * j + hb
            for u in range(2):
                nc.tensor.matmul(
                    (otB if hb else otA)[0:65, :],
                    Vx[:, 2 * g + u, h * 65 : h * 65 + 65],
                    pH[:, u, :],
                    start=(g == 0 and u == 0),
                    stop=(g == N_KT // 2 - 1 and u == 1),
                )

        def pair_slots(qc, j, slots, dve_slots=frozenset()):
            st = pair_state[(qc, j)]
            q_sl = slice(qc * 512, (qc + 1) * 512)
            for slot in slots:
                g, hb = slot // 2, slot % 2
                r_sl = slice(64, 128) if hb else slice(0, 64)
                sH = sp.tile([128, 2, 512], f32, tag="sAB", name=f"s{qc}{j}{slot}")
                for u in range(2):
                    kt = 2 * g + u
                    nc.tensor.matmul(
                        sH[:, u, :],
                        KT[r_sl, j, kt * 128 : (kt + 1) * 128],
                        QT[r_sl, j, q_sl],
                        start=True, stop=True,
                        tile_position=(64 * hb, 0),
                    )
                pH = ptp.tile([128, 2, 512], bf16, tag="pt", name=f"p{qc}{j}{slot}")
                if slot in dve_slots:
                    ti = ptp.tile(
                        [128, 2, 512], i32, tag="ti", bufs=3, name=f"ti{qc}{j}{slot}"
                    )
                    nc.vector.tensor_scalar(
                        ti[:], sH[:], float(SCALE * EXP_A), EXP_B,
                        mybir.AluOpType.mult, mybir.AluOpType.add,
                    )
                    nc.vector.tensor_copy(pH[:], ti[:].bitcast(f32))
                else:
                    nc.scalar.activation(
                        out=pH[:], in_=sH[:], func=Exp, scale=float(SCALE)
                    )
                st[2].append((slot, pH))
                while len(st[2]) > PV_LAG:
                    pslot, ppH = st[2].pop(0)
                    emit_pv(qc, j, pslot, ppH)

        def pair_end(qc, j, OT):
            st = pair_state[(qc, j)]
            for pkt, ppAB in st[2]:
                emit_pv(qc, j, pkt, ppAB)
            otA, otB, _ = pair_state.pop((qc, j))
            # 1/l rows straight from PSUM (ahead of the evac copies: the
            # broadcast DMA chain is the long pole into the final emit_y)
            rbs = rbp.tile([128, 2, 512], f32, tag="rbs", name=f"rbs{qc}{j}")
            nc.vector.reciprocal(out=rbs[64:65, 0, :], in_=otA[64:65, :])
            nc.vector.reciprocal(out=rbs[64:65, 1, :], in_=otB[64:65, :])
            # evacuate O^T from PSUM so the next pair's PVs can start
            # (B first: the obBh partition-move DMA depends on it)
            obB = obp.tile([128, 512], f32, tag="obB", name=f"obB{qc}{j}")
            nc.vector.tensor_copy(obB[0:64, :], otB[0:64, :])
            obA = obp.tile([128, 512], f32, tag="obA", name=f"obA{qc}{j}")
            nc.vector.tensor_copy(obA[0:64, :], otA[0:64, :])
            pi = qc * 4 + j
            nc.sync.dma_start(out=rb_dr.ap()[pi, :, :], in_=rbs[64:65, :, :])
            rbb = rbp.tile([128, 512], f32, tag="rbb", name=f"rbb{qc}{j}")
            nc.sync.dma_start(
                out=rbb[0:64, :], in_=rb_dr.ap()[pi, 0:1, :].broadcast_to([64, 512])
            )
            nc.sync.dma_start(
                out=rbb[64:128, :], in_=rb_dr.ap()[pi, 1:2, :].broadcast_to([64, 512])
            )
            # head B rows must land at partitions 64:128 of OT: move via DMA
            obBh = obp.tile([128, 512], f32, tag="obBh", name=f"obBh{qc}{j}")
            nc.sync.dma_start(out=obBh[64:128, :], in_=obB[0:64, :])
            nc.vector.tensor_mul(OT[0:64, j, :], obA[0:64, :], rbb[0:64, :])
            nc.vector.tensor_mul(OT[64:128, j, :], obBh[64:128, :], rbb[64:128, :])

        def emit_y(qc, OT, split_q=False):
            for st in range(4):
                yp = pp.tile([128, 512], f32, tag="proj")
                for dt in range(N_DT):
                    nc.tensor.matmul(
                        yp[:],
                        OT[:, dt, st * 128 : (st + 1) * 128],
                        woutT[:, dt, :],
                        start=(dt == 0),
                        stop=(dt == N_DT - 1),
                    )
                ys = ysp.tile([128, 512], f32, tag="ysb")
                nc.vector.tensor_add(ys[:], yp[:], bias_sb[:])
                (nc.scalar if (split_q and st % 2) else nc.sync).dma_start(
                    out=y_out.ap()[qc * 512 + st * 128 : qc * 512 + (st + 1) * 128, :],
                    in_=ys[:],
                )

        OT_tiles = {}
        OT_tiles[0] = otp.tile([128, N_DT, 512], bf16, tag="OT0", name="OT0")
        OT_tiles[1] = otp.tile([128, N_DT, 512], bf16, tag="OT1", name="OT1")

        # --- startup: weights + Q projection, x chunks prefetching ---
        cbw0 = load_cast(w_qkv, 0, q=nc.sync)
        cbq0 = load_cast(x_q, 0, q=nc.scalar, eng=nc.vector)
        cbx = {0: load_cast(x_full, 0, q=nc.scalar)}
        transpose_chunk(wqkvT, cbw0, 0)
        xqT0 = xtp.tile([128, N_DT, 512], bf16, tag="xqT")
        transpose_chunk(xqT0, cbq0)
        qproj(0, xqT0)
        cbx[1] = load_cast(x_full, 512, q=nc.scalar)
        cbw1 = load_cast(w_qkv, 512, q=nc.sync)
        transpose_chunk(wqkvT, cbw1, 512)
        cbq1 = load_cast(x_q, 512, q=nc.sync, eng=nc.vector)
        xqT1 = xtp.tile([128, N_DT, 512], bf16, tag="xqT")
        transpose_chunk(xqT1, cbq1)
        qproj(1, xqT1)
        cbw2 = load_cast(w_qkv, 1024, q=nc.sync)
        transpose_chunk(wqkvT, cbw2, 1024)
        xT = {0: xtp.tile([128, N_DT, 512], bf16, tag="xT", name="xT0")}
        transpose_chunk(xT[0], cbx[0])

        # ones columns of Vx (needed first at the initial PV, ~15us in;
        # emitted here so they queue behind the startup cast copies on Pool)
        for h in range(H):
            nc.gpsimd.memset(Vx[:, :, h * 65 + 64 : h * 65 + 65], 1.0)

        # --- K/V projection interleaved with the first attention pair ---
        pair_begin(0, 0)
        for sc in range(S // 512):
            if sc == 1:  # w_out chain deferred out of the startup critical path
                cbwo = load_cast(w_out, 0, q=nc.sync, eng=nc.vector)
                transpose_chunk(woutT, cbwo)
            if sc + 2 < S // 512:
                cbx[sc + 2] = load_cast(
                    x_full, (sc + 2) * 512, q=(nc.scalar if sc % 2 else nc.sync)
                )
            for et in range(N_DT):
                ps = pp.tile([128, 512], f32, tag="proj")
                for dt in range(N_DT):
                    nc.tensor.matmul(
                        ps[:],
                        wqkvT[:, dt, DIM + et * 128 : DIM + (et + 1) * 128],
                        xT[sc][:, dt, :],
                        start=(dt == 0),
                        stop=(dt == N_DT - 1),
                    )
                nc.vector.tensor_copy(KT[:, et, sc * 512 : (sc + 1) * 512], ps[:])
            for a in range(4):
                ps = pp.tile([128, 512], f32, tag="proj")
                for dt in range(N_DT):
                    nc.tensor.matmul(
                        ps[:],
                        xT[sc][:, dt, a * 128 : (a + 1) * 128],
                        wqkvT[:, dt, 2 * DIM : 3 * DIM],
                        start=(dt == 0),
                        stop=(dt == N_DT - 1),
                    )
                # scatter the 8 head slices into Vx (65-wide per head)
                nc.vector.tensor_copy(
                    Vx[:, sc * 4 + a, :]
                    .rearrange("p (h c) -> p h c", h=H)[:, :, 0:DH],
                    ps[:].rearrange("p (h c) -> p h c", h=H),
                )
            # attention on pair (qc=0, j=0) for the 4 k-tiles just produced
            pair_slots(0, 0, [4 * sc, 4 * sc + 1, 4 * sc + 2, 4 * sc + 3])
            if sc + 1 < S // 512:
                xT[sc + 1] = xtp.tile(
                    [128, N_DT, 512], bf16, tag="xT", name=f"xT{sc + 1}"
                )
                transpose_chunk(xT[sc + 1], cbx[sc + 1])
        pair_end(0, 0, OT_tiles[0])

        # --- remaining pairs (exp split across ACT + DVE) ---
        for qc, j in [(1, 0), (0, 1), (1, 1), (0, 2), (1, 2), (0, 3), (1, 3)]:
            pair_begin(qc, j)
            pair_slots(qc, j, list(range(N_KT)), dve_slots=DVE_SLOTS)
            pair_end(qc, j, OT_tiles[qc])
            if (qc, j) == (0, 3):
                emit_y(0, OT_tiles[0])
        emit_y(1, OT_tiles[1], split_q=True)

    nc.compile()
    return nc


def _get_nc():
    if "nc" not in _CACHE:
        _CACHE["nc"] = _build_program()
    return _CACHE["nc"]


def sim_time_estimate():
    """CoreSim cost-model span for one core with zero-filled inputs."""
    from concourse.bass_interp import CoreSim

    nc = _get_nc()
    sim = CoreSim(nc, publish_trace=False)
    sim.tensor("x_full")[:] = 0
    sim.tensor("x_q")[:] = 0
    sim.tensor("w_qkv")[:] = 0
    sim.tensor("w_out")[:] = 0
    sim.tensor("b_out")[:] = 0
    sim.simulate()
    return int(sim.time)


def kernel(x, w_qkv, w_out, b_out):
    from concourse.bass_utils import run_bass_kernel_spmd

    nc = _get_nc()
    x = np.asarray(x, dtype=np.float32)
    w_qkv = np.ascontiguousarray(np.asarray(w_qkv, dtype=np.float32))
    w_out = np.ascontiguousarray(np.asarray(w_out, dtype=np.float32))
    b_out = np.ascontiguousarray(np.asarray(b_out, dtype=np.float32)).reshape(1, DIM)

    in_maps = []
    for c in range(N_CORES):
        b = c // 4
        qo = (c % 4) * QLOC
        in_maps.append(
            {
                "x_full": np.ascontiguousarray(x[b]),
                "x_q": np.ascontiguousarray(x[b, qo : qo + QLOC]),
                "w_qkv": w_qkv,
                "w_out": w_out,
                "b_out": b_out,
            }
        )
    res = run_bass_kernel_spmd(nc, in_maps, list(range(N_CORES)))
    y = np.empty((B, S, DIM), dtype=np.float32)
    for c in range(N_CORES):
        b = c // 4
        qo = (c % 4) * QLOC
        y[b, qo : qo + QLOC] = res.results[c]["y"]
    return y



# revision 36
# speedup vs baseline: 1.2485x; 1.2485x over previous
# Multi-head attention (B=2, S=4096, D=512, H=8) on 8 Trainium2 NeuronCores.
#
# Sharding: core c handles batch b=c//4 and query rows [(c%4)*1024, (c%4+1)*1024).
# Each core computes K/V projections for its full batch element (duplicated
# across the 4 cores sharing a batch; avoids all cross-core communication),
# Q for its own slice, full 8-head attention for its query rows, and the
# output projection for its rows. The full output is the disjoint
# concatenation of the 8 per-core results.
#
# The host rotates each core's x_full copy so that the core's own query rows
# come first (softmax over keys is order-invariant, so the rotated key order
# changes nothing): Q projection reuses the first two transposed x chunks
# instead of loading/casting/transposing a separate x_q.
#
# On-core dataflow (matmuls bf16 with fp32 PSUM accumulation; fp8 was
# measured 2.26x faster for PV via DoubleRow but the e4m3 quantization of
# P or V alone costs ~1.8e-2 max-rel error on concentrated-attention rows,
# which blows the 2e-2 budget - so everything stays bf16):
#   x,W --HWDGE load--> SBUF f32 --Pool/DVE cast--> SBUF bf16
#     --PE transpose (identity built via iota)--> x^T, W^T tiles in SBUF
#   Q^T = Wq^T-tiles x x^T   K^T = Wk^T-tiles x x^T   V = x^T-tiles x Wv^T
#   V is stored with a ones-column appended per head (65 cols/head), so the
#   PV matmul produces both O^T (64 rows) and the softmax denominator l
#   (row 64) in one instruction -- no separate ones-matmul.
#   per (q-chunk 512, head-pair j), per k-tile kt of 128:
#     S^T[k,q] for BOTH heads -> one 2-bank PSUM tile   (2 row-packed MMs)
#     P^T = exp(S^T * scale): one ACT instruction for both heads, or a
#       Schraudolph bitcast-exp on DVE for a subset of k-tiles (splits the
#       exp throughput across two engines; the softmax denominator uses the
#       same approximated P so the ~3% sawtooth error cancels to 1st order)
#     O^T[0:65,q] += Vx-tile^T x P^T  per head (separate PSUM banks, m=65)
#   PV for k-tile kt is emitted PV_LAG slots behind its scores so the PE
#   never stalls behind the exp engines in its FIFO.
#   PSUM: one shared 3-deep ring of 2-bank slots serves projections,
#   transposes AND the score tiles, so the attention slot pipeline holds 3
#   score buffers and the PE can run ~1.5 slots ahead of the exp engines.
#   pair end: DVE copies O^T+l out of PSUM (frees banks fast), DVE reciprocal
#   of l, broadcast of 1/l across partitions via a small DRAM bounce, DVE
#   mul -> OT (bf16); head B's rows move to partitions 64:128 via SBUF DMA.
#   y = OT-tiles x Wout^T + b  (consumes OT directly as stationary operand)
import sys

if "/opt/trn_rl_repo" not in sys.path:
    sys.path.insert(0, "/opt/trn_rl_repo")

import numpy as np

B = 2
S = 4096
DIM = 512
H = 8
DH = DIM // H
SCALE = DH**-0.5
N_CORES = 8
QLOC = S // 4  # query rows per core
N_KT = S // 128  # k tiles of 128
N_DT = DIM // 128  # feature-dim tiles of 128
PV_LAG = 4  # slots between scores+exp emission and the PV matmul
# Slots (of 32 per pair) whose exp runs on DVE (Schraudolph bitcast exp).
# Balanced against ACT exp [128,2,512]->bf16 (~1272ns) vs DVE ts+bitcast-copy
# (~1160ns) with pair_end work (~2.6us) also on DVE: 17 ACT / 15 DVE,
# alternating so consecutive slots' exps overlap on different engines.
DVE_SLOTS = frozenset(range(2, 31, 2))
EXP_A = 12102203.161561485  # 2**23 / ln(2)
EXP_B = 1064986823.0  # 127 * 2**23 - 366393 (minimax shift)

_CACHE = {}


def _build_program(reps=1, ablate=None):
    # ablate: timing-only program variants for bottleneck attribution
    #   "p1"         = startup + phase-1 loop (with its pair) only
    #   "p1_noslots" = startup + phase-1 loop without attention slots
    #   "pairs2"     = full program but only 2 steady pairs
    from contextlib import ExitStack

    from concourse import bacc, mybir, tile

    f32 = mybir.dt.float32
    bf16 = mybir.dt.bfloat16
    i32 = mybir.dt.int32
    Exp = mybir.ActivationFunctionType.Exp
    ActCopy = mybir.ActivationFunctionType.Copy

    nc = bacc.Bacc("TRN2", target_bir_lowering=False, debug=False)

    x_full = nc.dram_tensor("x_full", [S, DIM], f32, kind="ExternalInput")
    w_qkv = nc.dram_tensor("w_qkv", [3 * DIM, DIM], f32, kind="ExternalInput")
    w_out = nc.dram_tensor("w_out", [DIM, DIM], f32, kind="ExternalInput")
    b_out = nc.dram_tensor("b_out", [1, DIM], f32, kind="ExternalInput")
    y_out = nc.dram_tensor("y", [QLOC, DIM], f32, kind="ExternalOutput")

    # per-pair 1/l rows bounced through DRAM to broadcast across partitions
    rb_dr = nc.dram_tensor("rb_dr", [8, 2, 512], f32)

    with tile.TileContext(nc) as tc, ExitStack() as ctx:
        if reps > 1:  # benchmarking only: repeat the whole body in a HW loop
            ctx.enter_context(tc.For_i(0, reps, 1))
        consts = ctx.enter_context(tc.tile_pool(name="consts", bufs=1))
        wp = ctx.enter_context(tc.tile_pool(name="wp", bufs=1))
        big = ctx.enter_context(tc.tile_pool(name="big", bufs=1))
        xtp = ctx.enter_context(tc.tile_pool(name="xtp", bufs=2))
        ptp = ctx.enter_context(tc.tile_pool(name="ptp", bufs=10))
        otp = ctx.enter_context(tc.tile_pool(name="otp", bufs=1))
        obp = ctx.enter_context(tc.tile_pool(name="obp", bufs=2))
        rbp = ctx.enter_context(tc.tile_pool(name="rbp", bufs=2))
        ysp = ctx.enter_context(tc.tile_pool(name="ysp", bufs=2))
        castp = ctx.enter_context(tc.tile_pool(name="castp", bufs=3))
        # PSUM: shared 3-deep ring of 2-bank slots for proj/transpose/
        # scores (6 banks) + otA + otB (2 banks) = 8
        pp = ctx.enter_context(tc.tile_pool(name="pp", bufs=3, space="PSUM"))
        op = ctx.enter_context(tc.tile_pool(name="op", bufs=1, space="PSUM"))

        bias_sb = consts.tile([128, DIM], f32, tag="bias")
        nc.gpsimd.dma_start(out=bias_sb[:], in_=b_out.ap().broadcast_to([128, DIM]))

        # identity for PE-mode transposes, built on-chip: (f - p) == 0
        it32 = consts.tile([128, 128], i32, tag="it32")
        nc.gpsimd.iota(it32[:], pattern=[[1, 128]], base=0, channel_multiplier=-1)
        ident = consts.tile([128, 128], bf16, tag="ident")
        nc.gpsimd.tensor_scalar(
            ident[:], it32[:], 0, None, mybir.AluOpType.is_equal
        )

        # --- persistent per-core tensors ---
        KT = big.tile([128, N_DT, S], bf16, tag="KT")  # K^T: part=(e-512)%128, [et, s]
        # V with a ones column appended per head: [k%128, kt, h*65+(dh|64)]
        Vx = big.tile([128, N_KT, H * (DH + 1)], bf16, tag="Vx")
        QT = big.tile([128, N_DT, QLOC], bf16, tag="QT")  # Q^T: part=e%128, [et, q]
        wqkvT = wp.tile([128, N_DT, 3 * DIM], bf16, tag="wqkvT")
        woutT = wp.tile([128, N_DT, DIM], bf16, tag="woutT")

        # --- load + cast (f32->bf16) a 512-row chunk into SBUF ---
        cast_n = [0]

        def load_cast(src_dram, r0, q=None, eng=None, rows=512):
            cast_n[0] += 1
            a = rows // 128
            xf = castp.tile([128, a, DIM], f32, tag="castf", name=f"cf{cast_n[0]}")
            (q or nc.sync).dma_start(
                out=xf[:],
                in_=src_dram.ap()[r0 : r0 + rows, :].rearrange(
                    "(a p) d -> p a d", p=128
                ),
            )
            xb = castp.tile([128, a, DIM], bf16, tag="castb", name=f"cb{cast_n[0]}")
            (eng or nc.gpsimd).tensor_copy(xb[:], xf[:])
            return xb

        # --- PE-transpose a cast chunk into a [128, dt, 512] T-layout tile ---
        # dst columns c0+a*128+c get rows r0+a*128+p of the source chunk.
        def transpose_chunk(dst, cb, c0=0, a0=0, na=4):
            for dt2 in range(N_DT // 2):
                tp = pp.tile([128, 2, 512], bf16, tag="proj", name=f"tp{cast_n[0]}{dt2}")
                for u in range(2):
                    dt = 2 * dt2 + u
                    for a in range(na):
                        nc.tensor.transpose(
                            tp[:, u, a * 128 : (a + 1) * 128],
                            cb[:, a, dt * 128 : (dt + 1) * 128],
                            ident[:],
                        )
                if na == 4:  # contiguous dst: one fused evac frees the ring faster
                    nc.vector.tensor_copy(
                        dst[:, 2 * dt2 : 2 * dt2 + 2, c0 : c0 + 512], tp[:]
                    )
                else:
                    for u in range(2):
                        dt = 2 * dt2 + u
                        nc.vector.tensor_copy(
                            dst[:, dt, c0 + a0 * 128 : c0 + (a0 + na) * 128],
                            tp[:, u, 0 : na * 128],
                        )

        def qproj(qc, xqT):
            for et in range(N_DT):
                ps = pp.tile([128, 512], f32, tag="proj")
                for dt in range(N_DT):
                    nc.tensor.matmul(
                        ps[:],
                        wqkvT[:, dt, et * 128 : (et + 1) * 128],
                        xqT[:, dt, :],
                        start=(dt == 0),
                        stop=(dt == N_DT - 1),
                    )
                nc.vector.tensor_copy(QT[:, et, qc * 512 : (qc + 1) * 512], ps[:])

        # --- attention machinery ---
        pair_state = {}

        def pair_begin(qc, j):
            otA = op.tile([128, 512], f32, tag="otA", name=f"otA{qc}{j}")
            otB = op.tile([128, 512], f32, tag="otB", name=f"otB{qc}{j}")
            pair_state[(qc, j)] = [otA, otB, []]

        def emit_pv(qc, j, slot, pH):
            # slot = 2*g + hb: head hb of pair j, k-tiles {2g, 2g+1}.
            # The 65-col stationary emits O^T (rows 0:64) plus the softmax
            # denominator (row 64).
            otA, otB, _ = pair_state[(qc, j)]
            g, hb = slot // 2, slot % 2
            h = 2 * j + hb
            for u in range(2):
                nc.tensor.matmul(
                    (otB if hb else otA)[0:65, :],
                    Vx[:, 2 * g + u, h * 65 : h * 65 + 65],
                    pH[:, u, :],
                    start=(g == 0 and u == 0),
                    stop=(g == N_KT // 2 - 1 and u == 1),
                )

        def pair_slots(qc, j, slots, dve_slots=frozenset()):
            # Emit the PE work in RUNS of a kind (8 score MMs, later 8 PV
            # MMs): alternating row-packed score MMs with full-array PV MMs
            # every 2 instructions was measured to serialize the PE ~2.5x
            # (config switches break the row-pack overlap); batching
            # recovers it (135us -> 38.7us per pair in isolation).
            st = pair_state[(qc, j)]
            q_sl = slice(qc * 512, (qc + 1) * 512)
            for b0 in range(0, len(slots), 4):
                made = []
                for slot in slots[b0 : b0 + 4]:
                    g, hb = slot // 2, slot % 2
                    r_sl = slice(64, 128) if hb else slice(0, 64)
                    sH = pp.tile(
                        [128, 2, 512], f32, tag="proj", name=f"s{qc}{j}{slot}"
                    )
                    for u in range(2):
                        kt = 2 * g + u
                        nc.tensor.matmul(
                            sH[:, u, :],
                            KT[r_sl, j, kt * 128 : (kt + 1) * 128],
                            QT[r_sl, j, q_sl],
                            start=True, stop=True,
                            tile_position=(64 * hb, 0),
                        )
                    made.append((slot, sH))
                for slot, sH in made:
                    pH = ptp.tile(
                        [128, 2, 512], bf16, tag="pt", name=f"p{qc}{j}{slot}"
                    )
                    if slot in dve_slots:
                        ti = ptp.tile(
                            [128, 2, 512], i32, tag="ti", bufs=3,
                            name=f"ti{qc}{j}{slot}",
                        )
                        nc.vector.tensor_scalar(
                            ti[:], sH[:], float(SCALE * EXP_A), EXP_B,
                            mybir.AluOpType.mult, mybir.AluOpType.add,
                        )
                        nc.vector.tensor_copy(pH[:], ti[:].bitcast(f32))
                    else:
                        nc.scalar.activation(
                            out=pH[:], in_=sH[:], func=Exp, scale=float(SCALE)
                        )
                    st[2].append((slot, pH))
                while len(st[2]) > PV_LAG:
                    n = len(st[2]) - PV_LAG
                    for _ in range(n):
                        pslot, ppH = st[2].pop(0)
                        emit_pv(qc, j, pslot, ppH)

        def pair_end(qc, j, OT):
            st = pair_state[(qc, j)]
            for pkt, ppAB in st[2]:
                emit_pv(qc, j, pkt, ppAB)
            otA, otB, _ = pair_state.pop((qc, j))
            # 1/l rows straight from PSUM (ahead of the evac copies: the
            # broadcast DMA chain is the long pole into the final emit_y)
            rbs = rbp.tile([128, 2, 512], f32, tag="rbs", name=f"rbs{qc}{j}")
            nc.vector.reciprocal(out=rbs[64:65, 0, :], in_=otA[64:65, :])
            nc.vector.reciprocal(out=rbs[64:65, 1, :], in_=otB[64:65, :])
            # evacuate O^T from PSUM so the next pair's PVs can start
            # (B first: the obBh partition-move DMA depends on it)
            obB = obp.tile([128, 512], f32, tag="obB", name=f"obB{qc}{j}")
            nc.vector.tensor_copy(obB[0:64, :], otB[0:64, :])
            obA = obp.tile([128, 512], f32, tag="obA", name=f"obA{qc}{j}")
            nc.vector.tensor_copy(obA[0:64, :], otA[0:64, :])
            pi = qc * 4 + j
            nc.sync.dma_start(out=rb_dr.ap()[pi, :, :], in_=rbs[64:65, :, :])
            rbb = rbp.tile([128, 512], f32, tag="rbb", name=f"rbb{qc}{j}")
            nc.sync.dma_start(
                out=rbb[0:64, :], in_=rb_dr.ap()[pi, 0:1, :].broadcast_to([64, 512])
            )
            nc.sync.dma_start(
                out=rbb[64:128, :], in_=rb_dr.ap()[pi, 1:2, :].broadcast_to([64, 512])
            )
            # head B rows must land at partitions 64:128 of OT: move via DMA
            obBh = obp.tile([128, 512], f32, tag="obBh", name=f"obBh{qc}{j}")
            nc.sync.dma_start(out=obBh[64:128, :], in_=obB[0:64, :])
            nc.vector.tensor_mul(OT[0:64, j, :], obA[0:64, :], rbb[0:64, :])
            nc.vector.tensor_mul(OT[64:128, j, :], obBh[64:128, :], rbb[64:128, :])

        def emit_y(qc, OT, split_q=False):
            for st in range(4):
                yp = pp.tile([128, 512], f32, tag="proj")
                for dt in range(N_DT):
                    nc.tensor.matmul(
                        yp[:],
                        OT[:, dt, st * 128 : (st + 1) * 128],
                        woutT[:, dt, :],
                        start=(dt == 0),
                        stop=(dt == N_DT - 1),
                    )
                ys = ysp.tile([128, 512], f32, tag="ysb")
                nc.vector.tensor_add(ys[:], yp[:], bias_sb[:])
                (nc.scalar if (split_q and st % 2) else nc.sync).dma_start(
                    out=y_out.ap()[qc * 512 + st * 128 : qc * 512 + (st + 1) * 128, :],
                    in_=ys[:],
                )

        OT_tiles = {}
        OT_tiles[0] = otp.tile([128, N_DT, 512], bf16, tag="OT0", name="OT0")
        OT_tiles[1] = otp.tile([128, N_DT, 512], bf16, tag="OT1", name="OT1")

        # --- startup: weights + Q projection from the first two x chunks ---
        xT = {}

        def new_xT(i):
            xT[i] = xtp.tile([128, N_DT, 512], bf16, tag="xT", name=f"xT{i}")

        cbw0 = load_cast(w_qkv, 0, q=nc.sync)
        cbx = {0: load_cast(x_full, 0, q=nc.scalar)}
        transpose_chunk(wqkvT, cbw0, 0)
        new_xT(0)
        transpose_chunk(xT[0], cbx[0])
        qproj(0, xT[0])
        cbx[1] = load_cast(x_full, 512, q=nc.scalar, eng=nc.vector)
        cbw1 = load_cast(w_qkv, 512, q=nc.sync)
        transpose_chunk(wqkvT, cbw1, 512)
        new_xT(1)
        transpose_chunk(xT[1], cbx[1])
        qproj(1, xT[1])
        cbw2 = load_cast(w_qkv, 1024, q=nc.sync)
        transpose_chunk(wqkvT, cbw2, 1024)

        # ones columns of Vx (needed first at the initial PV, ~15us in;
        # emitted here so they queue behind the startup cast copies on Pool)
        for h in range(H):
            nc.gpsimd.memset(Vx[:, :, h * 65 + 64 : h * 65 + 65], 1.0)

        # --- K/V projection interleaved with the first attention pair ---
        if ablate != "p1_noslots":
            pair_begin(0, 0)
        for sc in range(S // 512):
            if sc == 1:  # w_out chain deferred out of the startup critical path
                cbwo = load_cast(w_out, 0, q=nc.sync, eng=nc.vector)
                transpose_chunk(woutT, cbwo)
            if sc + 2 < S // 512 and sc + 2 >= 2:
                cbx[sc + 2] = load_cast(
                    x_full, (sc + 2) * 512, q=(nc.scalar if sc % 2 else nc.sync)
                )
            for et in range(N_DT):
                ps = pp.tile([128, 512], f32, tag="proj")
                for dt in range(N_DT):
                    nc.tensor.matmul(
                        ps[:],
                        wqkvT[:, dt, DIM + et * 128 : DIM + (et + 1) * 128],
                        xT[sc][:, dt, :],
                        start=(dt == 0),
                        stop=(dt == N_DT - 1),
                    )
                nc.vector.tensor_copy(KT[:, et, sc * 512 : (sc + 1) * 512], ps[:])
            for a in range(4):
                ps = pp.tile([128, 512], f32, tag="proj")
                for dt in range(N_DT):
                    nc.tensor.matmul(
                        ps[:],
                        xT[sc][:, dt, a * 128 : (a + 1) * 128],
                        wqkvT[:, dt, 2 * DIM : 3 * DIM],
                        start=(dt == 0),
                        stop=(dt == N_DT - 1),
                    )
                # scatter the 8 head slices into Vx (65-wide per head)
                nc.vector.tensor_copy(
                    Vx[:, sc * 4 + a, :]
                    .rearrange("p (h c) -> p h c", h=H)[:, :, 0:DH],
                    ps[:].rearrange("p (h c) -> p h c", h=H),
                )
            # attention on pair (qc=0, j=0) for the 4 k-tiles just produced
            if ablate != "p1_noslots":
                pair_slots(0, 0, [4 * sc, 4 * sc + 1, 4 * sc + 2, 4 * sc + 3])
            if sc + 1 < S // 512 and sc + 1 >= 2:
                new_xT(sc + 1)
                transpose_chunk(xT[sc + 1], cbx[sc + 1])
        if ablate != "p1_noslots":
            pair_end(0, 0, OT_tiles[0])

        # --- remaining pairs (exp split across ACT + DVE) ---
        # (0,3) third-from-last so emit_y(0) overlaps the last two pairs
        steady = [(1, 0), (0, 1), (1, 1), (0, 2), (0, 3), (1, 2), (1, 3)]
        if ablate in ("p1", "p1_noslots"):
            steady = []
        elif ablate == "pairs2":
            steady = [(1, 0), (0, 1)]
        for qc, j in steady:
            pair_begin(qc, j)
            pair_slots(qc, j, list(range(N_KT)), dve_slots=DVE_SLOTS)
            pair_end(qc, j, OT_tiles[qc])
        if ablate is None:
            emit_y(0, OT_tiles[0])
            emit_y(1, OT_tiles[1], split_q=True)

    nc.compile()
    return nc


def _get_nc():
    if "nc" not in _CACHE:
        _CACHE["nc"] = _build_program()
    return _CACHE["nc"]


def sim_time_estimate():
    """CoreSim cost-model span for one core with zero-filled inputs."""
    from concourse.bass_interp import CoreSim

    nc = _get_nc()
    sim = CoreSim(nc, publish_trace=False)
    sim.tensor("x_full")[:] = 0
    sim.tensor("w_qkv")[:] = 0
    sim.tensor("w_out")[:] = 0
    sim.tensor("b_out")[:] = 0
    sim.simulate()
    return int(sim.time)


def kernel(x, w_qkv, w_out, b_out):
    from concourse.bass_utils import run_bass_kernel_spmd

    nc = _get_nc()
    x = np.asarray(x, dtype=np.float32)
    w_qkv = np.ascontiguousarray(np.asarray(w_qkv, dtype=np.float32))
    w_out = np.ascontiguousarray(np.asarray(w_out, dtype=np.float32))
    b_out = np.ascontiguousarray(np.asarray(b_out, dtype=np.float32)).reshape(1, DIM)

    in_maps = []
    for c in range(N_CORES):
        b = c // 4
        qo = (c % 4) * QLOC
        in_maps.append(
            {
                # rotate so this core's query rows lead; softmax over keys is
                # order-invariant so the rotated K/V order is harmless
                "x_full": np.ascontiguousarray(np.roll(x[b], -qo, axis=0)),
                "w_qkv": w_qkv,
                "w_out": w_out,
                "b_out": b_out,
            }
        )
    res = run_bass_kernel_spmd(nc, in_maps, list(range(N_CORES)))
    y = np.empty((B, S, DIM), dtype=np.float32)
    for c in range(N_CORES):
        b = c // 4
        qo = (c % 4) * QLOC
        y[b, qo : qo + QLOC] = res.results[c]["y"]
    return y
